# revision 76
# baseline (speedup 1.0000x reference)
"""Trainium2 Bass kernel for the speech-enhancement loss function.

Math (matching the jax reference):
  loss_mag    = mean((clean_mag - enhan_mag)^2)
  d           = clean_pha - enhan_mag          (reference quirk: enhan_mag is phase_g)
  ip_loss     = mean(aw(d)),   aw(x) = |x - round(x/2pi)*2pi|
  gd_loss     = mean(aw(gd)),  gd[:,0,:] = -d[:,0,:]; gd[:,j,:] = d[:,j-1,:]-d[:,j,:]
  iaf_loss    = mean(aw(iaf)), same shifted difference along the T axis
  cspc_loss   = mean(1 - cos(aw(d))) = mean(1 - cos(d))
  loss_com    = mean((clean_com - enhan_com)^2) * 2
  loss_time   = mean(|clean_wav - enhan_wav|)
  loss_metric = mean((metric_g - 1)^2)            (tiny -> host)

Sharding: data-parallel over the batch dim, 2 batches per core on 8 cores.
Each core computes partial SUMS of each term into an SBUF accumulator
acc[128, NCOLS]; the full accumulator ships out and the host does the final
partition/column sums in float64.

The kernel is DMA-bound: 26.33 MB/core of fp32 input at the cost model's
360 B/ns DMA floor = 73.1 us. The design goal is every compute engine well
under that floor so the schedule is a single gapless DMA stream plus a
short tail. Engine busy (per core, of ~81 us): DVE ~61, ACT ~59, Pool ~43,
PE ~13.

Key tricks:
  * round(q) via the fp32 magic constant: v = q + 1.5*2^23; r = v - MAGIC.
    Both run on ACT as Identity(scale*x + bias) (exact fp32 in the interp),
    freeing DVE for d and f.
  * everything downstream of f = q - round(q) is bf16: DVE tensor_scalar
    cache-reduce ops with all-bf16 SBUF operands run at 4x (tensor_tensor
    at 2x), and the PE banded matmul for gd runs at 1 cycle/row instead of
    fp32's 4.
  * the anti-wrap distance dist(y) = |y| - 2*relu(|y|-0.5) avoids nested
    abs (this ISA has no DVE abs): for iaf it decomposes into four relu/
    identity cache-reduce sums on DVE (sum|y| = 2*sum relu(y) - sum y;
    relu(|y|-0.5) = relu(y-0.5) + relu(-y-0.5)); for gd, ACT's Abs from
    PSUM emits |y| (bf16) and accumulates sum|y| in one op, then two 4x
    DVE ops accumulate the relu term.
  * ip's sum|f| rides ACT Abs whose |f| output also feeds ACT Sin for
    cspc (cos(d) = sin(pi/2 - 2pi|f|), arg within [-pi/2, pi/2]).
  * dist is 1-periodic in y, so bf16 rounding of f (and round-boundary
    flips) shifts y by integers and cancels.
  * gd's cross-tile boundary row is patched by a K=128, 1-output-partition
    matmul (e127 selector) reading f_prev directly -- no SBUF->SBUF DMA,
    so SP.SEQ never blocks mid-stream.
  * per-term partial sums land in per-instruction accumulator columns
    (fp32, accum_out, 128 cols = full-rate 512B out descriptors); the host
    combines in float64, so no on-device final reduce sits on the tail.
Scheduling: all phase input DMAs issue first; each pass's d/v/r/f chain is
emitted ahead of the previous pass's accumulation block so buffer-freeing
ops lead every in-order queue. Com chunks follow (Pool sub + DVE square
pairs; the last 4 square on ACT to clear DVE's queue), wavs ride the
long-free phase-input tags mid-stream, then two DVE-sub tail chunks, and
the stream ends in three decomposed chunks: sum (c-e)^2 = sum c^2
- 2 sum ce + sum e^2, where c^2 squares on Pool (idle by then) with a 4x
DVE cache-reduce, and after the final ec transfer only sum ce (DVE) and
sum e^2 (ACT) remain, in parallel -- ~2.0 us from last byte to last
accum, plus the fixed ~3.2 us out-DMA/sem/drain chain. 80.5 us total vs
the 73.1 us DMA floor.
"""

import numpy as np

import concourse.bacc as bacc
import concourse.mybir as mybir
import concourse.tile as tile
from concourse.bass_utils import run_bass_kernel_spmd

F32 = mybir.dt.float32
BF16 = mybir.dt.bfloat16
OP = mybir.AluOpType
AF = mybir.ActivationFunctionType

B, F, T, L = 16, 201, 2048, 204800
NCORES = 8
BPC = B // NCORES  # batches per core

TWO_PI_64 = 2.0 * np.pi
S = float(np.float32(1.0) / np.float32(TWO_PI_64))  # 1/(2pi) in fp32
MAGIC = float(np.float32(1.5 * 2**23))  # 12582912.0, round-to-int trick
HALF_PI = float(np.float32(np.pi / 2))
NEG_TWO_PI = float(np.float32(-TWO_PI_64))

# com per core: BPC*F*T*2 = 1646592 = 2 batches x (128 x 6432)
COM_ROWS, COM_COLS = 128, 6432
COM_CHUNK = 1608  # 4 chunks per batch
# wav per core: BPC*L = 409600 = 128 x 3200
WAV_ROWS, WAV_COLS = 128, 3200

NCOLS = 128  # accumulator columns (128 x 4B = 512B rows: full-rate out DMA)

# term -> list of acc columns, populated by build_nc (deterministic)
COLMAP = {}


def _w0_matrix():
    # lhsT[k, j] = delta_{j,k+1} - delta_{j,k}  ->  (W0 @ f)[j] = f[j-1] - f[j]
    w = np.zeros((128, 128), dtype=np.float32)
    for k in range(128):
        w[k, k] = -1.0
        if k + 1 < 128:
            w[k, k + 1] = 1.0
    return w


def _e127_matrix():
    # lhsT[k, 0] = delta_{k,127}: a K=128, 1-output-partition matmul weight
    # that adds f_prev row 127 into output partition 0 -- boundary patch
    # without any SBUF->SBUF DMA.
    e = np.zeros((128, 1), dtype=np.float32)
    e[127, 0] = 1.0
    return e


def build_nc(
    nch=2,            # T-chunks per phase pass (pipeline the serial chain)
    in_bufs=3,        # cm/em double-buffer depth
    cp_bufs=2,        # cp buffer depth
    com_bufs=3,       # com/wav input buffer depth
    qg_chunks=2,      # gd PSUM halves
    cd_bufs=4,        # com diff-tile depth (Pool runs ahead of DVE's pairs)
    mag_dve=(),       # phase pass indices whose mag sub runs on DVE (else Pool)
    m2_act=(),        # phase pass indices whose m^2 sum runs on ACT Square
    wav_after=3,      # insert the two wav passes after this many com chunks
    sq_act_last=4,    # last N main com chunks square on ACT (clears DVE queue)
    tail_pre=(804, 804),        # DVE sub+stt tail chunks, DMA'd before wav
    dec_sizes=(737, 536, 335),  # final decomposed chunks (after wav)
    wav_dve=False,    # wav subs on DVE instead of Pool
    dec_split=0,      # DVE-side columns of the final chunk's sum e^2 (0 = all ACT)
):
    nc = bacc.Bacc(None, target_bir_lowering=False)

    mag_c = nc.dram_tensor("mag_c", [BPC, F, T], F32, kind="ExternalInput")
    mag_e = nc.dram_tensor("mag_e", [BPC, F, T], F32, kind="ExternalInput")
    pha_c = nc.dram_tensor("pha_c", [BPC, F, T], F32, kind="ExternalInput")
    com_c = nc.dram_tensor("com_c", [BPC, COM_ROWS, COM_COLS], F32, kind="ExternalInput")
    com_e = nc.dram_tensor("com_e", [BPC, COM_ROWS, COM_COLS], F32, kind="ExternalInput")
    wav_c = nc.dram_tensor("wav_c", [WAV_ROWS, WAV_COLS], F32, kind="ExternalInput")
    wav_e = nc.dram_tensor("wav_e", [WAV_ROWS, WAV_COLS], F32, kind="ExternalInput")
    out_d = nc.dram_tensor("partials", [128, NCOLS], F32, kind="ExternalOutput")



    COLMAP.clear()
    _next_col = [0]

    def col(term):
        c = _next_col[0]
        _next_col[0] += 1
        assert c < NCOLS
        COLMAP.setdefault(term, []).append(c)
        return c

    with tile.TileContext(nc) as tc:
        with (
            tc.tile_pool(name="main", bufs=2) as pool,
            tc.tile_pool(name="psum", bufs=1, space="PSUM") as psum,
        ):
            ftiles = [(0, 128), (128, 73)]  # (f0, P)
            f_prev_by_b = {}
            counters = {"pi": 0, "ci": 0, "wi": 0}
            phase_in = {}

            def emit_inputs(pi, b, f0, P):
                cm = pool.tile([P, T], F32, tag="in_a", bufs=in_bufs, name=f"cm{pi}")
                nc.sync.dma_start(cm[:], mag_c[b, f0 : f0 + P, :])
                em = pool.tile([P, T], F32, tag="in_b", bufs=in_bufs, name=f"em{pi}")
                nc.sync.dma_start(em[:], mag_e[b, f0 : f0 + P, :])
                cp = pool.tile([P, T], F32, tag="in_c", bufs=cp_bufs, name=f"cp{pi}")
                nc.sync.dma_start(cp[:], pha_c[b, f0 : f0 + P, :])
                phase_in[pi] = (cm, em, cp)

            def emit_consts():
                # w0 (banded shift matrix) and e127 (row-127 selector) are
                # built on-device via GPSIMD affine_select -- no DMA bytes in
                # the stream. Scratch tiles reuse later-phase tags.
                ones = pool.tile([128, 128], BF16, tag="cd", bufs=cd_bufs, name="cones")
                nc.vector.memset(ones[:], 1.0)
                sup = pool.tile([128, 128], BF16, tag="cd", bufs=cd_bufs, name="csup")
                nc.gpsimd.affine_select(
                    sup[:], ones[:], [[1, 128]], OP.is_equal, 0.0,
                    base=-1, channel_multiplier=-1,
                )
                diag = pool.tile([128, 128], BF16, tag="cd", bufs=cd_bufs, name="cdiag")
                nc.gpsimd.affine_select(
                    diag[:], ones[:], [[1, 128]], OP.is_equal, 0.0,
                    base=0, channel_multiplier=-1,
                )
                w0 = pool.tile([128, 128], BF16, tag="w0", bufs=1)
                nc.vector.tensor_tensor(w0[:], sup[:], diag[:], OP.subtract)
                e127 = pool.tile([128, 1], BF16, tag="e127", bufs=1)
                nc.gpsimd.affine_select(
                    e127[:], ones[:, 0:1], [[1, 1]], OP.is_equal, 0.0,
                    base=-127, channel_multiplier=1,
                )
                acc = pool.tile([128, NCOLS], F32, tag="acc", bufs=1)
                nc.vector.memset(acc[:], 0.0)
                halfpi = pool.tile([128, 1], F32, tag="halfpi", bufs=1)
                nc.vector.memset(halfpi[:], HALF_PI)
                magic = pool.tile([128, 1], F32, tag="magic", bufs=1)
                nc.vector.memset(magic[:], MAGIC)
                negmagic = pool.tile([128, 1], F32, tag="negmagic", bufs=1)
                nc.vector.memset(negmagic[:], -MAGIC)
                return w0, e127, acc, halfpi, magic, negmagic

            phase_state = {}

            def phase_chain(pi, b, f0, P):
                cm, em, cp = phase_in[pi]
                CT = T // (nch[pi] if isinstance(nch, (list, tuple)) else nch)
                d = pool.tile([P, T], F32, tag="d", name=f"d{pi}")
                v = pool.tile([P, T], F32, tag="v", name=f"v{pi}")
                f = pool.tile([P, T], BF16, tag="f", bufs=3, name=f"f{pi}")
                chunks = [slice(c0, c0 + CT) for c0 in range(0, T, CT)]

                # round chain: d (DVE) -> v, r (ACT, r in-place) -> f (DVE)
                for ts_ in chunks:
                    nc.vector.tensor_tensor(d[:, ts_], cp[:, ts_], em[:, ts_], OP.subtract)
                for ts_ in chunks:
                    nc.scalar.activation(
                        v[:, ts_], d[:, ts_], AF.Identity, bias=magic[0:P, :], scale=S
                    )
                for ts_ in chunks:
                    nc.scalar.activation(
                        v[:, ts_], v[:, ts_], AF.Identity, bias=negmagic[0:P, :]
                    )
                for ts_ in chunks:
                    nc.vector.scalar_tensor_tensor(
                        f[:, ts_], d[:, ts_], S, v[:, ts_], OP.mult, OP.subtract
                    )
                phase_state[pi] = (f, chunks)

            def phase_accums(pi, b, f0, P):
                f_prev = f_prev_by_b.get(b)
                cm, em, cp = phase_in[pi]
                f, chunks = phase_state[pi]
                af = pool.tile([P, T], BF16, tag="af", bufs=1, name=f"af{pi}")
                js = pool.tile([P, T], BF16, tag="js", bufs=1, name=f"js{pi}")
                fd = pool.tile([P, T], BF16, tag="fd", name=f"fd{pi}")
                at = pool.tile([P, T], BF16, tag="at", bufs=1, name=f"at{pi}")
                zt = pool.tile([P, T], BF16, tag="zt", bufs=1, name=f"zt{pi}")
                # ip: af = |f| on ACT with accum (feeds Sin); cspc: sin accum
                for ts_ in chunks:
                    nc.scalar.activation(
                        af[:, ts_], f[:, ts_], AF.Abs,
                        accum_out=acc[0:P, (c := col("ip")) : c + 1],
                    )
                for ts_ in chunks:
                    nc.scalar.activation(
                        js[:, ts_], af[:, ts_], AF.Sin, bias=halfpi[0:P, :],
                        scale=NEG_TWO_PI,
                        accum_out=acc[0:P, (c := col("cos")) : c + 1],
                    )
                # mag: m = cm - em (bf16); sum m^2: DVE tt-mult + cache-reduce
                # (in-place), or ACT Square+accum for m2_act passes
                m = pool.tile([P, T], BF16, tag="m", name=f"m{pi}")
                if pi in mag_dve:
                    nc.vector.tensor_tensor(m[:], cm[:], em[:], OP.subtract)
                else:
                    nc.gpsimd.tensor_tensor(m[:], cm[:], em[:], OP.subtract)
                if pi in m2_act:
                    mj = pool.tile([P, T], BF16, tag="mj", bufs=1, name=f"mj{pi}")
                    nc.scalar.activation(
                        mj[:], m[:], AF.Square,
                        accum_out=acc[0:P, (c := col("m2")) : c + 1],
                    )
                else:
                    z = pool.tile([P, T], BF16, tag="z", bufs=1, name=f"z{pi}")
                    nc.vector.tensor_tensor(z[:], m[:], m[:], OP.mult)
                    nc.vector.tensor_scalar(
                        z[:], z[:], 0.0, 0.0, OP.add, OP.add,
                        accum_out=acc[0:P, (c := col("m2")) : c + 1],
                    )
                # iaf: fd = shifted diff (DVE 2x); dist sums stay entirely on
                # DVE via relu identities (all 4x bf16 cache-reduce ops):
                #   sum|y| = 2*sum relu(y) - sum y
                #   sum relu(|y|-0.5) = sum relu(y-0.5) + sum relu(-y-0.5)
                # dist(y) = |y| - 2*relu(|y| - 0.5)
                for ts_ in chunks:
                    tc0 = ts_.start
                    lo = tc0 if tc0 else 1
                    if tc0 == 0:
                        nc.vector.tensor_copy(fd[:, 0:1], f[:, 0:1])
                    nc.vector.tensor_tensor(
                        fd[:, lo : ts_.stop], f[:, lo - 1 : ts_.stop - 1],
                        f[:, lo : ts_.stop], OP.subtract
                    )
                for ts_ in chunks:
                    nc.vector.tensor_scalar(
                        zt[:, ts_], fd[:, ts_], 0.0, 0.0, OP.max, OP.add,
                        accum_out=acc[0:P, (c := col("iafA")) : c + 1],
                    )
                    nc.vector.tensor_scalar(
                        zt[:, ts_], fd[:, ts_], 0.0, 0.0, OP.add, OP.add,
                        accum_out=acc[0:P, (c := col("iafB")) : c + 1],
                    )
                    nc.vector.tensor_scalar(zt[:, ts_], fd[:, ts_], -0.5, None, OP.add)
                    nc.vector.tensor_scalar(
                        at[:, ts_], zt[:, ts_], 0.0, 0.0, OP.max, OP.add,
                        accum_out=acc[0:P, (c := col("iafC")) : c + 1],
                    )
                    nc.vector.tensor_scalar(
                        zt[:, ts_], fd[:, ts_], -1.0, -0.5, OP.mult, OP.add
                    )
                    nc.vector.tensor_scalar(
                        at[:, ts_], zt[:, ts_], 0.0, 0.0, OP.max, OP.add,
                        accum_out=acc[0:P, (c := col("iafD")) : c + 1],
                    )
                # gd via PE banded mm (bf16, e127 patch); same relu-dist sums
                HT = T // qg_chunks
                for h in range(qg_chunks):
                    qg = psum.tile([P, HT], F32, tag="qg", bufs=2, name=f"qg{pi}_{h}")
                    for n0 in range(0, HT, 512):
                        nn = h * HT + n0
                        if f_prev is None:
                            nc.tensor.matmul(
                                qg[:, n0 : n0 + 512], w0[0:P, 0:P],
                                f[:, nn : nn + 512],
                            )
                        else:
                            nc.tensor.matmul(
                                qg[:, n0 : n0 + 512], w0[0:P, 0:P],
                                f[:, nn : nn + 512], start=True, stop=False,
                            )
                            nc.tensor.matmul(
                                qg[0:1, n0 : n0 + 512], e127[0:128, 0:1],
                                f_prev[:, nn : nn + 512], start=False, stop=True,
                            )
                    ag = pool.tile([P, HT], BF16, tag="ag", bufs=1, name=f"ag{pi}_{h}")
                    nc.scalar.activation(
                        ag[:], qg[:], AF.Abs,
                        accum_out=acc[0:P, (c := col("gda")) : c + 1],
                    )
                    gt = pool.tile([P, HT], BF16, tag="gt", name=f"gt{pi}_{h}")
                    nc.vector.tensor_scalar(gt[:], ag[:], -0.5, None, OP.add)
                    nc.vector.tensor_scalar(
                        gt[:], gt[:], 0.0, 0.0, OP.max, OP.add,
                        accum_out=acc[0:P, (c := col("gdr")) : c + 1],
                    )
                f_prev_by_b[b] = f

            def com_pass(b, c0, ck, mode="main", sq_act=False, alt=False):
                # tail chunks ride long-free tags (in_a/in_b for inputs, fd
                # for the diff) so their DMAs never wait on backlogged
                # queues; alternating tail chunks use the com tags to double
                # the effective input-buffer depth
                if mode == "main":
                    ta, tb, td = ("com_a", "com_b", "cd")
                elif alt:
                    ta, tb, td = ("com_a", "com_b", "fd")
                else:
                    ta, tb, td = ("in_a", "in_b", "fd")
                ci = counters["ci"]
                counters["ci"] += 1
                cc = pool.tile([COM_ROWS, ck], F32, tag=ta, bufs=com_bufs, name=f"cc{ci}")
                nc.sync.dma_start(cc[:], com_c[b, :, c0 : c0 + ck])
                ec = pool.tile([COM_ROWS, ck], F32, tag=tb, bufs=com_bufs, name=f"ec{ci}")
                nc.sync.dma_start(ec[:], com_e[b, :, c0 : c0 + ck])
                if mode == "tail_dec":
                    # sum (c-e)^2 = sum c^2 - 2 sum ce + sum e^2: sum c^2
                    # squares on Pool (idle by now) with a cheap 4x DVE
                    # cache-reduce; after the final ec only sum ce (DVE) and
                    # sum e^2 (ACT) remain, in parallel on different engines.
                    j1p = pool.tile([COM_ROWS, ck], BF16, tag="m", name=f"j1p_{ci}")
                    nc.gpsimd.tensor_tensor(j1p[:], cc[:], cc[:], OP.mult)
                    j1 = pool.tile([COM_ROWS, ck], BF16, tag="zt", bufs=1, name=f"j1_{ci}")
                    nc.vector.tensor_scalar(
                        j1[:], j1p[:], 0.0, 0.0, OP.add, OP.add,
                        accum_out=acc[:, (c := col("c2a")) : c + 1],
                    )
                    j3 = pool.tile([COM_ROWS, ck], BF16, tag="at", bufs=1, name=f"j3_{ci}")
                    nc.vector.scalar_tensor_tensor(
                        j3[:], cc[:], 0.0, ec[:], OP.bypass, OP.mult,
                        accum_out=acc[:, (c := col("c2m")) : c + 1],
                    )
                    j2 = pool.tile([COM_ROWS, ck], BF16, tag="js", bufs=1, name=f"j2_{ci}")
                    nc.scalar.activation(
                        j2[:], ec[:], AF.Square,
                        accum_out=acc[:, (c := col("c2b")) : c + 1],
                    )
                    return
                cd = pool.tile([COM_ROWS, ck], BF16, tag=td, bufs=2 if mode != "main" else cd_bufs, name=f"cd{ci}")
                if mode in ("tail_dve", "tail_da"):
                    nc.vector.tensor_tensor(cd[:], cc[:], ec[:], OP.subtract)
                else:
                    nc.gpsimd.tensor_tensor(cd[:], cc[:], ec[:], OP.subtract)
                if mode == "tail_da":
                    # DVE sub + ACT square: halves DVE's tail throughput load
                    cj = pool.tile([COM_ROWS, ck], BF16, tag="js", bufs=1, name=f"cjd{ci}")
                    nc.scalar.activation(
                        cj[:], cd[:], AF.Square,
                        accum_out=acc[:, (c := col("c2")) : c + 1],
                    )
                elif mode == "tail_dve":
                    # all-DVE: bf16 tt-mult (2x) + cache-reduce (4x)
                    cz = pool.tile([COM_ROWS, ck], BF16, tag="zt", bufs=1, name=f"cz{ci}")
                    nc.vector.tensor_tensor(cz[:], cd[:], cd[:], OP.mult)
                    nc.vector.tensor_scalar(
                        cz[:], cz[:], 0.0, 0.0, OP.add, OP.add,
                        accum_out=acc[:, (c := col("c2")) : c + 1],
                    )
                elif mode == "tail_pa" or sq_act:
                    cj = pool.tile([COM_ROWS, ck], BF16, tag="js" if mode == "tail_pa" else "wj", bufs=1, name=f"cja{ci}")
                    nc.scalar.activation(
                        cj[:], cd[:], AF.Square,
                        accum_out=acc[:, (c := col("c2")) : c + 1],
                    )
                else:
                    cz = pool.tile([COM_ROWS, ck], BF16, tag="cz", bufs=1, name=f"cz{ci}")
                    nc.vector.tensor_tensor(cz[:], cd[:], cd[:], OP.mult)
                    nc.vector.tensor_scalar(
                        cz[:], cz[:], 0.0, 0.0, OP.add, OP.add,
                        accum_out=acc[:, (c := col("c2")) : c + 1],
                    )

            def wav_pass(c0, ck, sub_dve=False):
                # wav rides the (long-free) phase-input tags so it never
                # couples with the com tile recycling
                wi = counters["wi"]
                counters["wi"] += 1
                cw = pool.tile([WAV_ROWS, ck], F32, tag="in_a", bufs=in_bufs, name=f"cw{wi}")
                nc.sync.dma_start(cw[:], wav_c[:, c0 : c0 + ck])
                ew = pool.tile([WAV_ROWS, ck], F32, tag="in_b", bufs=in_bufs, name=f"ew{wi}")
                nc.sync.dma_start(ew[:], wav_e[:, c0 : c0 + ck])
                wd = pool.tile([WAV_ROWS, ck], BF16, tag="m", name=f"wd{wi}")
                if sub_dve:
                    nc.vector.tensor_tensor(wd[:], cw[:], ew[:], OP.subtract)
                else:
                    nc.gpsimd.tensor_tensor(wd[:], cw[:], ew[:], OP.subtract)
                wj = pool.tile([WAV_ROWS, ck], BF16, tag="wj", bufs=1, name=f"wj{wi}")
                nc.scalar.activation(
                    wj[:], wd[:], AF.Abs,
                    accum_out=acc[:, (c := col("w")) : c + 1],
                )

            # ---- emission: input DMAs first (SP order = pure input stream),
            # then phase compute, then coms/wavs with a shrinking DVE tail ----
            plist = [(b, f0, P) for b in range(BPC) for f0, P in ftiles]
            emit_inputs(0, *plist[0])
            w0, e127, acc, halfpi, magic, negmagic = emit_consts()
            for pi in range(1, 4):
                emit_inputs(pi, *plist[pi])
            # interleave: chain(k+1) is emitted before accums(k) so d/f ops
            # (which recycle input buffers and feed ACT) lead every engine
            # queue; the accumulation blocks fill the gaps behind them
            phase_chain(0, *plist[0])
            phase_chain(1, *plist[1])
            phase_accums(0, *plist[0])
            phase_chain(2, *plist[2])
            phase_accums(1, *plist[1])
            phase_chain(3, *plist[3])
            phase_accums(2, *plist[2])
            phase_accums(3, *plist[3])

            # com chunks (Pool subs) with wavs woven mid-stream for slack,
            # then the DVE-sub tail chunks, and the lean decomposed chunks
            n_tail = sum(tail_pre) + sum(dec_sizes)
            main_chunks = [(0, c0) for c0 in range(0, COM_COLS, COM_CHUNK)] + [
                (1, c0) for c0 in range(0, COM_COLS - n_tail, COM_CHUNK)
            ]
            for i, (b, c0) in enumerate(main_chunks):
                com_pass(b, c0, COM_CHUNK,
                         sq_act=(i >= len(main_chunks) - sq_act_last))
                if i == wav_after - 1:
                    wav_pass(0, 1600, sub_dve=wav_dve)
                    wav_pass(1600, 1600, sub_dve=wav_dve)
            c0 = COM_COLS - n_tail
            for j, ck in enumerate(tail_pre):
                com_pass(1, c0, ck, mode="tail_dve")
                c0 += ck
            # decomposed tail: sum (c-e)^2 = sum c^2 - 2 sum ce + sum e^2.
            # Chunk 0's c^2 runs directly on DVE (free then); later chunks'
            # c^2 square on Pool with their DVE cache-reduces DEFERRED past
            # all sum-ce ops so the Pool handoff never blocks DVE's queue.
            nd = len(dec_sizes)
            for j, ck in enumerate(dec_sizes):
                cc = pool.tile([COM_ROWS, ck], F32, tag="in_a", bufs=com_bufs, name=f"dcc{j}")
                nc.sync.dma_start(cc[:], com_c[1, :, c0 : c0 + ck])
                ec = pool.tile([COM_ROWS, ck], F32, tag="in_b", bufs=com_bufs, name=f"dec{j}")
                nc.sync.dma_start(ec[:], com_e[1, :, c0 : c0 + ck])
                if j == 0:
                    j1 = pool.tile([COM_ROWS, ck], BF16, tag="zt", bufs=1, name=f"dj1_{j}")
                    nc.vector.scalar_tensor_tensor(
                        j1[:], cc[:], 0.0, cc[:], OP.bypass, OP.mult,
                        accum_out=acc[:, (c := col("c2a")) : c + 1],
                    )
                    j1p = None
                else:
                    j1p = pool.tile([COM_ROWS, ck], BF16, tag="m", name=f"dj1p_{j}")
                    nc.gpsimd.tensor_tensor(j1p[:], cc[:], cc[:], OP.mult)
                j3 = pool.tile([COM_ROWS, ck], BF16, tag="at", bufs=1, name=f"dj3_{j}")
                nc.vector.scalar_tensor_tensor(
                    j3[:], cc[:], 0.0, ec[:], OP.bypass, OP.mult,
                    accum_out=acc[:, (c := col("c2m")) : c + 1],
                )
                if j == nd - 1 and dec_split:
                    # split the final sum e^2 across ACT and DVE to balance
                    # the two engines' post-last-byte finish times
                    h = ck - dec_split
                    j2 = pool.tile([COM_ROWS, h], BF16, tag="js", bufs=1, name=f"dj2_{j}")
                    nc.scalar.activation(
                        j2[:], ec[:, 0:h], AF.Square,
                        accum_out=acc[:, (c := col("c2b")) : c + 1],
                    )
                else:
                    j2 = pool.tile([COM_ROWS, ck], BF16, tag="js", bufs=1, name=f"dj2_{j}")
                    nc.scalar.activation(
                        j2[:], ec[:], AF.Square,
                        accum_out=acc[:, (c := col("c2b")) : c + 1],
                    )
                if j1p is not None:
                    # deferred cache-reduce of Pool's c^2, right after this
                    # chunk's sum-ce so it never blocks the next chunk
                    j1 = pool.tile([COM_ROWS, ck], BF16, tag="zt", bufs=1, name=f"dj1_{j}")
                    nc.vector.tensor_scalar(
                        j1[:], j1p[:], 0.0, 0.0, OP.add, OP.add,
                        accum_out=acc[:, (c := col("c2a")) : c + 1],
                    )
                if j == nd - 1 and dec_split:
                    j2d = pool.tile([COM_ROWS, dec_split], BF16, tag="zt", bufs=1, name="dj2d")
                    nc.vector.scalar_tensor_tensor(
                        j2d[:], ec[:, ck - dec_split : ck], 0.0,
                        ec[:, ck - dec_split : ck], OP.bypass, OP.mult,
                        accum_out=acc[:, (c := col("c2b")) : c + 1],
                    )
                c0 += ck

            # ---- ship the whole accumulator; host reduces ----
            nc.sync.dma_start(out_d[:], acc[:])

    nc.compile()
    return nc


_CACHE = {}


def _get_nc():
    if "nc" not in _CACHE:
        _CACHE["nc"] = build_nc()
    return _CACHE["nc"]


def make_in_maps(inputs):
    """Slice the full inputs into per-core input maps."""
    clean_mag = np.asarray(inputs["clean_mag"], dtype=np.float32)
    enhan_mag = np.asarray(inputs["enhan_mag"], dtype=np.float32)
    clean_pha = np.asarray(inputs["clean_pha"], dtype=np.float32)
    clean_com = np.asarray(inputs["clean_com"], dtype=np.float32)
    enhan_com = np.asarray(inputs["enhan_com"], dtype=np.float32)
    clean_wav = np.asarray(inputs["clean_wav"], dtype=np.float32)
    enhan_wav = np.asarray(inputs["enhan_wav"], dtype=np.float32)

    in_maps = []
    for i in range(NCORES):
        sl = slice(BPC * i, BPC * (i + 1))
        in_maps.append(
            {
                "mag_c": np.ascontiguousarray(clean_mag[sl]),
                "mag_e": np.ascontiguousarray(enhan_mag[sl]),
                "pha_c": np.ascontiguousarray(clean_pha[sl]),
                "com_c": np.ascontiguousarray(clean_com[sl]).reshape(
                    BPC, COM_ROWS, COM_COLS
                ),
                "com_e": np.ascontiguousarray(enhan_com[sl]).reshape(
                    BPC, COM_ROWS, COM_COLS
                ),
                "wav_c": np.ascontiguousarray(clean_wav[sl]).reshape(
                    WAV_ROWS, WAV_COLS
                ),
                "wav_e": np.ascontiguousarray(enhan_wav[sl]).reshape(
                    WAV_ROWS, WAV_COLS
                ),
            }
        )
    return in_maps


def combine(partials, inputs):
    """Combine per-core partials ([NCORES, 128, NCOLS]) into the 6 losses."""
    p = np.asarray(partials, dtype=np.float64)
    p = p.reshape(-1, NCOLS).sum(axis=0)

    def tsum(term):
        return sum(p[c] for c in COLMAP[term])

    s_ip = tsum("ip")
    s_cos = tsum("cos")
    s_m2 = tsum("m2")
    s_c2 = tsum("c2")
    s_w = tsum("w")

    n = float(B * F * T)
    ip = TWO_PI_64 * s_ip / n
    # gd/iaf: device cols hold sum|y| and sum relu(|y|-0.5);
    # dist(y) = |y| - 2*relu(|y|-0.5)
    gd = TWO_PI_64 * (tsum("gda") - 2.0 * tsum("gdr")) / n
    s_iaf_abs = 2.0 * tsum("iafA") - tsum("iafB")
    iaf = TWO_PI_64 * (
        s_iaf_abs - 2.0 * (tsum("iafC") + tsum("iafD"))
    ) / n
    cspc = 1.0 - s_cos / n
    loss_mag = s_m2 / n
    loss_pha = ip + gd + iaf + cspc
    if "c2a" in COLMAP:
        s_c2 += tsum("c2a") + tsum("c2b") - 2.0 * tsum("c2m")
    loss_com = 2.0 * s_c2 / (n * 2.0)
    loss_time = s_w / float(B * L)

    metric_g = np.asarray(inputs["metric_g"], dtype=np.float64).reshape(-1)
    one_labels = np.asarray(inputs["one_labels"], dtype=np.float64).reshape(-1)
    loss_metric = float(np.mean((metric_g - one_labels) ** 2))

    nloss = (
        loss_mag * 0.9
        + loss_pha * 0.3
        + loss_com * 0.1
        + loss_metric * 0.05
        + loss_time * 0.2
    )
    return tuple(
        np.float32(x)
        for x in (nloss, loss_mag, loss_pha, loss_com, loss_metric, loss_time)
    )


def _get_runner():
    """Build (once) a persistently-compiled 8-core sharded executor.

    Mirrors bass2jax.run_bass_via_pjrt but caches the jitted function so
    repeat calls skip retracing/recompiling. Returns
    (call(concat_inputs) -> partials[NCORES, 128, NCOLS], in_names,
    device_put_fn).
    """
    if "runner" in _CACHE:
        return _CACHE["runner"]
    import jax
    from concourse import bass2jax

    nc = _get_nc()
    bass2jax.install_neuronx_cc_hook()

    partition_name = nc.partition_id_tensor.name if nc.partition_id_tensor else None
    in_names, out_names, out_avals, zero_shapes = [], [], [], []
    for alloc in nc.m.functions[0].allocations:
        if not isinstance(alloc, mybir.MemoryLocationSet):
            continue
        name = alloc.memorylocations[0].name
        if alloc.kind == "ExternalInput":
            if name != partition_name:
                in_names.append(name)
        elif alloc.kind == "ExternalOutput":
            out_names.append(name)
            shape = tuple(alloc.tensor_shape)
            dtype = mybir.dt.np(alloc.dtype)
            out_avals.append(jax.core.ShapedArray(shape, dtype))
            zero_shapes.append((shape, dtype))
    n_params = len(in_names)
    all_in = list(in_names) + list(out_names)
    if partition_name is not None:
        all_in.append(partition_name)
    donate = tuple(range(n_params, n_params + len(out_names)))

    def _body(*args):
        operands = list(args)
        if partition_name is not None:
            operands.append(bass2jax.partition_id_tensor())
        outs = bass2jax._bass_exec_p.bind(
            *operands,
            out_avals=tuple(out_avals),
            in_names=tuple(all_in),
            out_names=tuple(out_names),
            lowering_input_output_aliases=(),
            sim_require_finite=True,
            sim_require_nnan=True,
            nc=nc,
        )
        return tuple(outs)

    devices = jax.devices()[:NCORES]
    mesh = bass2jax.Mesh(np.asarray(devices), ("core",))
    pspec = bass2jax.PartitionSpec("core")
    in_specs = (pspec,) * (n_params + len(out_names))
    out_specs = (pspec,) * len(out_names)
    sharded = jax.jit(
        bass2jax.shard_map(
            _body, mesh=mesh, in_specs=in_specs, out_specs=out_specs, check_rep=False
        ),
        donate_argnums=donate,
        keep_unused=True,
    )

    def make_zeros():
        return [
            np.zeros((NCORES * s[0], *s[1:]), d) for (s, d) in zero_shapes
        ]

    def call(concat_in):
        outs = sharded(*concat_in, *make_zeros())
        return np.asarray(outs[0]).reshape(NCORES, 128, NCOLS)

    def device_put(concat_in):
        sh = jax.sharding.NamedSharding(mesh, pspec)
        return [jax.device_put(a, sh) for a in concat_in]

    runner = (call, in_names, device_put, sharded, make_zeros)
    _CACHE["runner"] = runner
    return runner


def concat_inputs(in_maps, in_names):
    return [
        np.concatenate([m[name] for m in in_maps], axis=0) for name in in_names
    ]


def run(inputs):
    in_maps = make_in_maps(inputs)
    try:
        call, in_names, _, _, _ = _get_runner()
        partials = call(concat_inputs(in_maps, in_names))
    except Exception:
        nc = _get_nc()
        res = run_bass_kernel_spmd(nc, in_maps, core_ids=list(range(NCORES)))
        partials = np.asarray([r["partials"] for r in res.results])
    return combine(partials, inputs)


def kernel(**inputs):
    return run(inputs)
